# revision 11
# baseline (speedup 1.0000x reference)
"""Trainium2 Bass kernel for CrossAttention.

Problem shape (hardcoded):
  latent  [8, 4096, 512], context [8, 77, 768]
  wq [512,512], wk/wv [768,512], wo [512,512], biases [512]
  out = softmax((latent@wq+bq)(context@wk+bk)^T / 8) @ (context@wv+bv) @ wo + bo

Sharding: data-parallel over batch — core b handles batch element b.

Key perf structure vs the naive version:
  * 256-row iterations: every PE matmul streams N=256..512 so the ~56ns
    LDWEIGHTS pipelines under the matmul stream (LD(i+1) || MM(i)).
  * Q is computed head-PAIR-major: qT_g [128, 256] = wq_pair^T @ xT with
    M=128 (full PE width).  Scores for heads 2g,2g+1 both consume qT_g with
    per-head zero-masked kT (top/bottom 64 partitions), so K stays 128.
  * kT free dim extended to 128 with zero cols -> score rows 77..127 are
    exactly 0, exp gives 1.0 there, and ones/v masks (zero rows >=77) kill
    them in the sums/PV contractions.  No per-iter zero fills anywhere.
  * Biases: bq via DVE tensor_scalar add (per-partition), bk via ACT bias,
    bv via in-PSUM e0 matmul (rows <77 only), bo via e0 matmul.
  * softmax normalize: single DVE tensor_tensor divide attnT_ps / sums_ps.
  * x^T via XBAR DMA transpose (SBUF->SBUF, idle DMA hw) or PE transpose.
  * optional fp8e4 DoubleRow for q-proj and out-proj (2 K-tiles/instr).

Engine budget per 256-row iter: PE ~40 matmuls (all N>=256), ACT = exp
only, DVE = qT bias-copy + divide, GPSIMD = SWDGE x cast-load + out copy.
"""

import os
import sys
from contextlib import ExitStack

import numpy as np

for _p in ("/opt/trn_rl_repo",):
    if _p not in sys.path and os.path.isdir(_p):
        sys.path.insert(0, _p)

import concourse.bass as bass  # noqa: E402
import concourse.tile as tile  # noqa: E402
from concourse import bacc, mybir  # noqa: E402
from concourse.bass_utils import run_bass_kernel_spmd  # noqa: E402
from concourse.masks import make_identity  # noqa: E402

N_CORES = 8
SQ, D, DC, SKV, H, DH = 4096, 512, 768, 77, 8, 64
F32 = mybir.dt.float32
BF16 = mybir.dt.bfloat16
FP8 = mybir.dt.float8e4
AF = mybir.ActivationFunctionType
ALU = mybir.AluOpType
DR = mybir.MatmulPerfMode.DoubleRow

# ---- config flags (A/B-able via env) ----
QP_FP8 = os.environ.get("CA_QP", "bf16") == "fp8"   # q-projection dtype
OP_FP8 = os.environ.get("CA_OP", "bf16") == "fp8"   # out-projection dtype
XT_MODE = os.environ.get("CA_XT", "pe")
USE_DIV = os.environ.get("CA_DIV", "recip") == "div"
USE_GPS = os.environ.get("CA_GPS", "0") == "1"   # gpsimd compute ops
NROW = 256                       # rows per iteration
NCH = NROW // 128                # 128-row chunks per iteration

SC_WQ = 256.0 if QP_FP8 else 1.0     # wq prescale (fp8 subnormal avoidance)
SC_Q = 1.0 / SC_WQ                   # q descale on PSUM->SBUF
SC_V = 32.0 if OP_FP8 else 1.0       # v prescale
SC_WO = 256.0 if OP_FP8 else 1.0     # wo prescale
SC_OUT = 1.0 / (SC_V * SC_WO)        # out descale
X_DT = FP8 if QP_FP8 else BF16
A_DT = FP8 if OP_FP8 else BF16       # attnT dtype into out-proj


def build_nc(n_iters=SQ // NROW):
    nc = bacc.Bacc("TRN2", target_bir_lowering=False, debug=False)

    lat = nc.dram_tensor("latent", [SQ, D], F32, kind="ExternalInput").ap()
    ctx_d = nc.dram_tensor("context", [SKV, DC], F32, kind="ExternalInput").ap()
    wq = nc.dram_tensor("wq", [D, D], F32, kind="ExternalInput").ap()
    bq = nc.dram_tensor("bq", [D], F32, kind="ExternalInput").ap()
    wk = nc.dram_tensor("wk", [DC, D], F32, kind="ExternalInput").ap()
    bk = nc.dram_tensor("bk", [D], F32, kind="ExternalInput").ap()
    wv = nc.dram_tensor("wv", [DC, D], F32, kind="ExternalInput").ap()
    bv = nc.dram_tensor("bv", [D], F32, kind="ExternalInput").ap()
    wo = nc.dram_tensor("wo", [D, D], F32, kind="ExternalInput").ap()
    bo = nc.dram_tensor("bo", [D], F32, kind="ExternalInput").ap()
    out_d = nc.dram_tensor("out", [SQ, D], F32, kind="ExternalOutput").ap()

    with tile.TileContext(nc) as tc:
        with ExitStack() as stk:
            consts = stk.enter_context(tc.tile_pool(name="consts", bufs=1))
            prep = stk.enter_context(tc.tile_pool(name="prep", bufs=1))
            xpool = stk.enter_context(tc.tile_pool(name="x", bufs=3))
            spool = stk.enter_context(tc.tile_pool(name="work", bufs=2))
            opool = stk.enter_context(tc.tile_pool(name="outp", bufs=3))
            # PSUM: psq tag "qT" slots hold both qT [128,256]f32 and out
            # [128,512]f32 tiles (1 bank x 2 bufs); pss 2 banks; psw 4 banks.
            psq = stk.enter_context(tc.tile_pool(name="psq", bufs=2, space="PSUM"))
            pss = stk.enter_context(tc.tile_pool(name="pss", bufs=2, space="PSUM"))
            psw = stk.enter_context(tc.tile_pool(name="psw", bufs=1, space="PSUM"))

            # ---------------- constants ----------------
            # weights (in,out) rearranged (t p) d -> p t d ; SWDGE casts f32->bf16
            wq_sb = consts.tile([128, 4, D], BF16, name="wq_sb")
            nc.gpsimd.dma_start(wq_sb, wq.rearrange("(t p) d -> p t d", p=128))
            wk_sb = consts.tile([128, 6, D], BF16, name="wk_sb")
            nc.gpsimd.dma_start(wk_sb, wk.rearrange("(t p) d -> p t d", p=128))
            wv_sb = consts.tile([128, 6, D], BF16, name="wv_sb")
            nc.gpsimd.dma_start(wv_sb, wv.rearrange("(t p) d -> p t d", p=128))
            wo_sb = consts.tile([128, 4, D], BF16, name="wo_sb")
            nc.gpsimd.dma_start(wo_sb, wo.rearrange("(t p) d -> p t d", p=128))
            if QP_FP8:
                wq_mm = consts.tile([128, 4, D], FP8, name="wq_mm")
                nc.vector.tensor_scalar_mul(wq_mm, wq_sb, SC_WQ)
            else:
                wq_mm = wq_sb
            if OP_FP8:
                wo_mm = consts.tile([128, 4, D], FP8, name="wo_mm")
                nc.vector.tensor_scalar_mul(wo_mm, wo_sb, SC_WO)
            else:
                wo_mm = wo_sb

            # bq pair-major: partition p, col g  <-> bq[g*128 + p]
            bq_pair = consts.tile([128, 4], F32, name="bq_pair")
            nc.sync.dma_start(bq_pair, bq.rearrange("(g p) -> p g", p=128))
            # bk head-major halves, prescaled by 1/8: partition q*64+d, col g
            # <-> bk[(2g+q)*64 + d] * 0.125
            bk_hm = consts.tile([128, 4], F32, name="bk_hm")
            nc.sync.dma_start(bk_hm, bk.rearrange("(g q d) -> (q d) g", g=4, q=2))
            bk_hms = consts.tile([128, 4], F32, name="bk_hms")
            nc.vector.tensor_scalar_mul(bk_hms, bk_hm, 0.125)

            # row-0 padded biases for rank-1 e0 matmuls
            bv_pad = consts.tile([128, D], BF16, name="bv_pad")
            nc.vector.memset(bv_pad, 0.0)
            nc.gpsimd.dma_start(bv_pad[0:1, :], bv.rearrange("(o d) -> o d", o=1))
            if USE_GPS:
                # bo replicated across partitions for the fused gpsimd copy
                bo_bcast = consts.tile([128, D], F32, name="bo_bcast")
                nc.sync.dma_start(
                    bo_bcast[0:1, :], bo.rearrange("(o d) -> o d", o=1)
                )
                nc.gpsimd.partition_broadcast(bo_bcast, bo_bcast[0:1, :])
            else:
                # row-0 padded bo for the rank-1 e0 bias matmul
                bo_pad = consts.tile([128, D], BF16, name="bo_pad")
                nc.vector.memset(bo_pad, 0.0)
                nc.gpsimd.dma_start(
                    bo_pad[0:1, :], bo.rearrange("(o d) -> o d", o=1)
                )
            # e0: row 0 all-ones
            e0 = consts.tile([128, 128], BF16, name="e0")
            nc.vector.memset(e0, 0.0)
            nc.vector.memset(e0[0:1, :], 1.0)
            # ones on kv rows < 77 (softmax-sum lhsT), zero padding rows
            ones64 = consts.tile([128, DH], BF16, name="ones64")
            nc.vector.memset(ones64, 0.0)
            nc.vector.memset(ones64[:SKV, :], 1.0)
            ident = consts.tile([128, 128], BF16, name="ident")
            make_identity(nc, ident)
            if QP_FP8:
                ident_x = consts.tile([128, 128], FP8, name="ident_x")
                make_identity(nc, ident_x)
            else:
                ident_x = ident

            # ---------------- K/V prep (once) ----------------
            ctx_sb = prep.tile([128, DC], BF16, name="ctx_sb")
            nc.vector.memset(ctx_sb, 0.0)
            nc.gpsimd.dma_start(ctx_sb[:SKV, :], ctx_d)
            # cT [128, 6, 128]; cols >= 77 are transposed zero rows -> 0
            cT_sb = prep.tile([128, 6, 128], BF16, name="cT_sb")
            for g in range(2):
                cT_ps = pss.tile([128, 3, 128], BF16, tag="sT", name="cT_ps")
                for t3 in range(3):
                    t = g * 3 + t3
                    nc.tensor.transpose(
                        cT_ps[:, t3, :], ctx_sb[:, t * 128 : (t + 1) * 128], ident
                    )
                nc.vector.tensor_copy(cT_sb[:, 3 * g : 3 * g + 3, :], cT_ps)

            # kT per head, masked into pair halves:
            #   head h lives in partitions (h%2)*64..+64 of kT_sb[:, h, :];
            #   the other 64 partitions are zero; cols 77..127 zero.
            kT_sb = prep.tile([128, H, 128], BF16, name="kT_sb")
            nc.vector.memset(kT_sb, 0.0)
            for g in range(4):
                kT_ps = psq.tile([128, 128], F32, tag="qT", name="kT_ps")
                for q in range(2):
                    h = 2 * g + q
                    off = q * 64
                    for ct in range(6):
                        nc.tensor.matmul(
                            kT_ps[off : off + 64, :],
                            lhsT=wk_sb[:, ct, h * 64 : (h + 1) * 64],
                            rhs=cT_sb[:, ct, :],
                            start=(ct == 0),
                            stop=(ct == 5),
                        )
                for q in range(2):
                    h = 2 * g + q
                    off = q * 64
                    nc.scalar.activation(
                        kT_sb[off : off + 64, h, :SKV],
                        kT_ps[off : off + 64, :SKV],
                        AF.Identity,
                        bias=bk_hms[off : off + 64, g : g + 1],
                        scale=0.125,
                    )

            # v [kv, d] with rows >= 77 exactly 0 (zero cT cols + e0[:, :77])
            v_ps = psq.tile([128, D], F32, tag="qT", name="v_ps")
            for ct in range(6):
                nc.tensor.matmul(
                    v_ps,
                    lhsT=cT_sb[:, ct, :],
                    rhs=wv_sb[:, ct, :],
                    start=(ct == 0),
                    stop=False,
                )
            nc.tensor.matmul(v_ps, lhsT=e0, rhs=bv_pad, start=False, stop=True)
            # rows >= 77 got bv from the rank-1 bias matmul; zero them (they
            # must mask the exp(0)=1 padding rows in the PV contraction).
            v_sb = prep.tile([128, D], BF16, name="v_sb")
            nc.vector.memset(v_sb[64:128, :], 0.0)
            nc.vector.tensor_scalar_mul(v_sb[:SKV, :], v_ps[:SKV, :], SC_V)

            # ---------------- main loop ----------------
            def do_iter(it):
                r0 = it * NROW
                # --- load x (SWDGE cast) + transpose ---
                x_sb = xpool.tile([128, NCH, D], X_DT, tag="x", name="x_sb")
                nc.gpsimd.dma_start(
                    x_sb, lat[r0 : r0 + NROW, :].rearrange("(c p) d -> p c d", p=128)
                )
                xT_sb = xpool.tile([128, 4, NROW], X_DT, tag="xT", name="xT_sb")
                if XT_MODE == "xbar":
                    for c in range(NCH):
                        nc.sync.dma_start_transpose(
                            xT_sb[:, :, c * 128 : (c + 1) * 128], x_sb[:, c, :]
                        )
                else:
                    for c in range(NCH):
                        xT_ps = psq.tile([128, 4, 128], X_DT, tag="qT", name="xT_ps")
                        for et in range(4):
                            nc.tensor.transpose(
                                xT_ps[:, et, :],
                                x_sb[:, c, et * 128 : (et + 1) * 128],
                                ident_x,
                            )
                        nc.scalar.copy(xT_sb[:, :, c * 128 : (c + 1) * 128], xT_ps)

                # --- q projection, head-pair-major ---
                qT_sb = spool.tile([128, 4, NROW], BF16, tag="qT", name="qT_sb")
                for g in range(4):
                    qT_ps = psq.tile([128, NROW], F32, tag="qT", name="qT_ps")
                    if QP_FP8:
                        for e2 in range(2):
                            nc.tensor.matmul(
                                qT_ps,
                                lhsT=wq_mm[:, 2 * e2 : 2 * e2 + 2, g * 128 : (g + 1) * 128],
                                rhs=xT_sb[:, 2 * e2 : 2 * e2 + 2, :],
                                start=(e2 == 0),
                                stop=(e2 == 1),
                                perf_mode=DR,
                            )
                    else:
                        for et in range(4):
                            nc.tensor.matmul(
                                qT_ps,
                                lhsT=wq_mm[:, et, g * 128 : (g + 1) * 128],
                                rhs=xT_sb[:, et, :],
                                start=(et == 0),
                                stop=(et == 3),
                            )
                    nc.vector.tensor_scalar(
                        qT_sb[:, g, :], qT_ps, SC_Q, bq_pair[:, g : g + 1],
                        ALU.mult, ALU.add,
                    )

                # --- scores + exp (per head pair) ---
                expT_sb = spool.tile([128, H, NROW], BF16, tag="expT", name="expT_sb")
                for g in range(4):
                    sT_ps = pss.tile([128, 2, NROW], F32, tag="sT", name="sT_ps")
                    for q in range(2):
                        nc.tensor.matmul(
                            sT_ps[:, q, :],
                            lhsT=kT_sb[:, 2 * g + q, :],
                            rhs=qT_sb[:, g, :],
                            start=True,
                            stop=True,
                        )
                    nc.scalar.activation(
                        expT_sb[:, 2 * g : 2 * g + 2, :], sT_ps, AF.Exp
                    )

                # --- sums + PV (pair-packed halves) ---
                sums_ps = psw.tile([128, 4, NROW], F32, tag="sums", name="sums_ps")
                attnT_ps = psw.tile([128, 4, NROW], F32, tag="attnT", name="attnT_ps")
                for h in range(H):
                    dt, off = h // 2, (h % 2) * 64
                    nc.tensor.matmul(
                        sums_ps[off : off + 64, dt, :],
                        lhsT=ones64,
                        rhs=expT_sb[:, h, :],
                        start=True,
                        stop=True,
                    )
                for h in range(H):
                    dt, off = h // 2, (h % 2) * 64
                    nc.tensor.matmul(
                        attnT_ps[off : off + 64, dt, :],
                        lhsT=v_sb[:, h * 64 : (h + 1) * 64],
                        rhs=expT_sb[:, h, :],
                        start=True,
                        stop=True,
                    )
                attnT_sb = spool.tile([128, 4, NROW], A_DT, tag="attnT", name="attnT_sb")
                if USE_DIV:
                    nc.vector.tensor_tensor(
                        attnT_sb, attnT_ps, sums_ps, ALU.divide
                    )
                else:
                    rsum_sb = spool.tile([128, 4, NROW], F32, tag="rsum", name="rsum_sb")
                    nc.vector.reciprocal_approx_fast(rsum_sb, sums_ps)
                    nc.vector.tensor_mul(attnT_sb, attnT_ps, rsum_sb)

                # --- out projection + store ---
                for c in range(NCH):
                    out_ps = psq.tile([128, D], F32, tag="qT", name="out_ps")
                    if OP_FP8:
                        for d2 in range(2):
                            nc.tensor.matmul(
                                out_ps,
                                lhsT=attnT_sb[:, 2 * d2 : 2 * d2 + 2, c * 128 : (c + 1) * 128],
                                rhs=wo_mm[:, 2 * d2 : 2 * d2 + 2, :],
                                start=(d2 == 0),
                                stop=(d2 == 1),
                                perf_mode=DR,
                            )
                    else:
                        for dt in range(4):
                            nc.tensor.matmul(
                                out_ps,
                                lhsT=attnT_sb[:, dt, c * 128 : (c + 1) * 128],
                                rhs=wo_mm[:, dt, :],
                                start=(dt == 0),
                                stop=(USE_GPS and dt == 3),
                            )
                    out_sb = opool.tile([128, D], F32, tag="out", name="out_sb")
                    if USE_GPS:
                        nc.gpsimd.tensor_add(out_sb, out_ps, bo_bcast)
                    else:
                        nc.tensor.matmul(
                            out_ps, lhsT=e0, rhs=bo_pad, start=False, stop=True
                        )
                        nc.scalar.copy(out_sb, out_ps)
                    nc.sync.dma_start(
                        out_d[r0 + c * 128 : r0 + (c + 1) * 128, :], out_sb
                    )

            for it in range(n_iters):
                do_iter(it)

    nc.compile()
    return nc


_BUILD_CACHE = {}


def _get_nc():
    key = (QP_FP8, OP_FP8, XT_MODE, USE_DIV)
    if key not in _BUILD_CACHE:
        _BUILD_CACHE[key] = build_nc()
    return _BUILD_CACHE[key]


def _in_maps(latent, context, wq, bq, wk, bk, wv, bv, wo, bo):
    f = lambda a: np.ascontiguousarray(np.asarray(a), dtype=np.float32)
    shared = {
        "wq": f(wq), "bq": f(bq), "wk": f(wk), "bk": f(bk),
        "wv": f(wv), "bv": f(bv), "wo": f(wo), "bo": f(bo),
    }
    maps = []
    for b in range(N_CORES):
        m = dict(shared)
        m["latent"] = f(latent[b])
        m["context"] = f(context[b])
        maps.append(m)
    return maps


def run_on_hw(inputs, trace=False, **kw):
    nc = _get_nc()
    maps = _in_maps(**inputs)
    res = run_bass_kernel_spmd(nc, maps, list(range(N_CORES)), trace=trace, **kw)
    out = np.stack([res.results[b]["out"] for b in range(N_CORES)], axis=0)
    return out, res


def kernel(latent, context, wq, bq, wk, bk, wv, bv, wo, bo):
    out, _ = run_on_hw(dict(
        latent=latent, context=context, wq=wq, bq=bq, wk=wk, bk=bk,
        wv=wv, bv=bv, wo=wo, bo=bo,
    ))
    return out


# revision 12
# speedup vs baseline: 1.5698x; 1.5698x over previous
"""Trainium2 Bass kernel for CrossAttention.

Problem shape (hardcoded):
  latent  [8, 4096, 512], context [8, 77, 768]
  wq [512,512], wk/wv [768,512], wo [512,512], biases [512]
  out = softmax((latent@wq+bq)(context@wk+bk)^T / 8) @ (context@wv+bv) @ wo + bo

Sharding: data-parallel over batch — core b handles batch element b.

Key perf structure vs the naive version:
  * 256-row iterations: every PE matmul streams N=256..512 so the ~56ns
    LDWEIGHTS pipelines under the matmul stream (LD(i+1) || MM(i)).
  * Q is computed head-PAIR-major: qT_g [128, 256] = wq_pair^T @ xT with
    M=128 (full PE width).  Scores for heads 2g,2g+1 both consume qT_g with
    per-head zero-masked kT (top/bottom 64 partitions), so K stays 128.
  * kT free dim extended to 128 with zero cols -> score rows 77..127 are
    exactly 0, exp gives 1.0 there, and ones/v masks (zero rows >=77) kill
    them in the sums/PV contractions.  No per-iter zero fills anywhere.
  * Biases: bq via DVE tensor_scalar add (per-partition), bk via ACT bias,
    bv via in-PSUM e0 matmul (rows <77 only), bo via e0 matmul.
  * softmax normalize: single DVE tensor_tensor divide attnT_ps / sums_ps.
  * x^T via XBAR DMA transpose (SBUF->SBUF, idle DMA hw) or PE transpose.
  * optional fp8e4 DoubleRow for q-proj and out-proj (2 K-tiles/instr).

Engine budget per 256-row iter: PE ~40 matmuls (all N>=256), ACT = exp
only, DVE = qT bias-copy + divide, GPSIMD = SWDGE x cast-load + out copy.
"""

import os
import sys
from contextlib import ExitStack

import numpy as np

for _p in ("/opt/trn_rl_repo",):
    if _p not in sys.path and os.path.isdir(_p):
        sys.path.insert(0, _p)

import concourse.bass as bass  # noqa: E402
import concourse.tile as tile  # noqa: E402
from concourse import bacc, mybir  # noqa: E402
from concourse.bass_utils import run_bass_kernel_spmd  # noqa: E402
from concourse.masks import make_identity  # noqa: E402

N_CORES = 8
SQ, D, DC, SKV, H, DH = 4096, 512, 768, 77, 8, 64
F32 = mybir.dt.float32
BF16 = mybir.dt.bfloat16
FP8 = mybir.dt.float8e4
AF = mybir.ActivationFunctionType
ALU = mybir.AluOpType
DR = mybir.MatmulPerfMode.DoubleRow

# ---- config flags (A/B-able via env) ----
QP_FP8 = os.environ.get("CA_QP", "bf16") == "fp8"   # q-projection dtype
OP_FP8 = os.environ.get("CA_OP", "bf16") == "fp8"   # out-projection dtype
XT_MODE = os.environ.get("CA_XT", "pe")
USE_DIV = os.environ.get("CA_DIV", "recip") == "div"
USE_GPS = os.environ.get("CA_GPS", "0") == "1"   # gpsimd compute ops
NROW = 256                       # rows per iteration
NCH = NROW // 128                # 128-row chunks per iteration

SC_WQ = 256.0 if QP_FP8 else 1.0     # wq prescale (fp8 subnormal avoidance)
SC_Q = 1.0 / SC_WQ                   # q descale on PSUM->SBUF
SC_V = 32.0 if OP_FP8 else 1.0       # v prescale
SC_WO = 256.0 if OP_FP8 else 1.0     # wo prescale
SC_OUT = 1.0 / (SC_V * SC_WO)        # out descale
X_DT = FP8 if QP_FP8 else BF16
A_DT = FP8 if OP_FP8 else BF16       # attnT dtype into out-proj


def build_nc(n_iters=SQ // NROW):
    nc = bacc.Bacc("TRN2", target_bir_lowering=False, debug=False)

    lat = nc.dram_tensor("latent", [SQ, D], F32, kind="ExternalInput").ap()
    ctx_d = nc.dram_tensor("context", [SKV, DC], F32, kind="ExternalInput").ap()
    wq = nc.dram_tensor("wq", [D, D], F32, kind="ExternalInput").ap()
    bq = nc.dram_tensor("bq", [D], F32, kind="ExternalInput").ap()
    wk = nc.dram_tensor("wk", [DC, D], F32, kind="ExternalInput").ap()
    bk = nc.dram_tensor("bk", [D], F32, kind="ExternalInput").ap()
    wv = nc.dram_tensor("wv", [DC, D], F32, kind="ExternalInput").ap()
    bv = nc.dram_tensor("bv", [D], F32, kind="ExternalInput").ap()
    wo = nc.dram_tensor("wo", [D, D], F32, kind="ExternalInput").ap()
    bo = nc.dram_tensor("bo", [D], F32, kind="ExternalInput").ap()
    out_d = nc.dram_tensor("out", [SQ, D], F32, kind="ExternalOutput").ap()

    with tile.TileContext(nc) as tc:
        with ExitStack() as stk:
            consts = stk.enter_context(tc.tile_pool(name="consts", bufs=1))
            prep = stk.enter_context(tc.tile_pool(name="prep", bufs=1))
            xpool = stk.enter_context(tc.tile_pool(name="x", bufs=3))
            spool = stk.enter_context(tc.tile_pool(name="work", bufs=2))
            opool = stk.enter_context(tc.tile_pool(name="outp", bufs=3))
            # PSUM (8 banks): psq tags "qx" (xT+qT, 2x1 bank) + "out"
            # (2x1 bank); pss tag "sT" (scores+sums ring, 2x1 bank);
            # psw "attnT" (2 banks, bufs=1).
            psq = stk.enter_context(tc.tile_pool(name="psq", bufs=2, space="PSUM"))
            pss = stk.enter_context(tc.tile_pool(name="pss", bufs=2, space="PSUM"))
            psw = stk.enter_context(tc.tile_pool(name="psw", bufs=1, space="PSUM"))

            # ---------------- constants ----------------
            # weights (in,out) rearranged (t p) d -> p t d ; SWDGE casts f32->bf16
            wq_sb = consts.tile([128, 4, D], BF16, name="wq_sb")
            nc.gpsimd.dma_start(wq_sb, wq.rearrange("(t p) d -> p t d", p=128))
            wk_sb = consts.tile([128, 6, D], BF16, name="wk_sb")
            nc.gpsimd.dma_start(wk_sb, wk.rearrange("(t p) d -> p t d", p=128))
            wv_sb = consts.tile([128, 6, D], BF16, name="wv_sb")
            nc.gpsimd.dma_start(wv_sb, wv.rearrange("(t p) d -> p t d", p=128))
            wo_sb = consts.tile([128, 4, D], BF16, name="wo_sb")
            nc.gpsimd.dma_start(wo_sb, wo.rearrange("(t p) d -> p t d", p=128))
            if QP_FP8:
                wq_mm = consts.tile([128, 4, D], FP8, name="wq_mm")
                nc.vector.tensor_scalar_mul(wq_mm, wq_sb, SC_WQ)
            else:
                wq_mm = wq_sb
            if OP_FP8:
                wo_mm = consts.tile([128, 4, D], FP8, name="wo_mm")
                nc.vector.tensor_scalar_mul(wo_mm, wo_sb, SC_WO)
            else:
                wo_mm = wo_sb

            # bq pair-major: partition p, col g  <-> bq[g*128 + p]
            bq_pair = consts.tile([128, 4], F32, name="bq_pair")
            nc.sync.dma_start(bq_pair, bq.rearrange("(g p) -> p g", p=128))
            # bk head-major halves, prescaled by 1/8: partition q*64+d, col g
            # <-> bk[(2g+q)*64 + d] * 0.125
            bk_hm = consts.tile([128, 4], F32, name="bk_hm")
            nc.sync.dma_start(bk_hm, bk.rearrange("(g q d) -> (q d) g", g=4, q=2))
            bk_hms = consts.tile([128, 4], F32, name="bk_hms")
            nc.vector.tensor_scalar_mul(bk_hms, bk_hm, 0.125)

            # row-0 padded biases for rank-1 e0 matmuls
            bv_pad = consts.tile([128, D], BF16, name="bv_pad")
            nc.vector.memset(bv_pad, 0.0)
            nc.gpsimd.dma_start(bv_pad[0:1, :], bv.rearrange("(o d) -> o d", o=1))
            if USE_GPS:
                # bo replicated across partitions for the fused gpsimd copy
                bo_bcast = consts.tile([128, D], F32, name="bo_bcast")
                nc.sync.dma_start(
                    bo_bcast[0:1, :], bo.rearrange("(o d) -> o d", o=1)
                )
                nc.gpsimd.partition_broadcast(bo_bcast, bo_bcast[0:1, :])
            else:
                # row-0 padded bo for the rank-1 e0 bias matmul
                bo_pad = consts.tile([128, D], BF16, name="bo_pad")
                nc.vector.memset(bo_pad, 0.0)
                nc.gpsimd.dma_start(
                    bo_pad[0:1, :], bo.rearrange("(o d) -> o d", o=1)
                )
            # e0: row 0 all-ones
            e0 = consts.tile([128, 128], BF16, name="e0")
            nc.vector.memset(e0, 0.0)
            nc.vector.memset(e0[0:1, :], 1.0)
            # ones on kv rows < 77 (softmax-sum lhsT), zero padding rows
            ones64 = consts.tile([128, DH], BF16, name="ones64")
            nc.vector.memset(ones64, 0.0)
            nc.vector.memset(ones64[:SKV, :], 1.0)
            ident = consts.tile([128, 128], BF16, name="ident")
            make_identity(nc, ident)
            if QP_FP8:
                ident_x = consts.tile([128, 128], FP8, name="ident_x")
                make_identity(nc, ident_x)
            else:
                ident_x = ident

            # ---------------- K/V prep (once) ----------------
            ctx_sb = prep.tile([128, DC], BF16, name="ctx_sb")
            nc.vector.memset(ctx_sb, 0.0)
            nc.gpsimd.dma_start(ctx_sb[:SKV, :], ctx_d)
            # cT [128, 6, 128]; cols >= 77 are transposed zero rows -> 0
            cT_sb = prep.tile([128, 6, 128], BF16, name="cT_sb")
            for g in range(2):
                cT_ps = pss.tile([128, 3, 128], BF16, tag="sT", name="cT_ps")
                for t3 in range(3):
                    t = g * 3 + t3
                    nc.tensor.transpose(
                        cT_ps[:, t3, :], ctx_sb[:, t * 128 : (t + 1) * 128], ident
                    )
                nc.vector.tensor_copy(cT_sb[:, 3 * g : 3 * g + 3, :], cT_ps)

            # kT per head, masked into pair halves:
            #   head h lives in partitions (h%2)*64..+64 of kT_sb[:, h, :];
            #   the other 64 partitions are zero; cols 77..127 zero.
            kT_sb = prep.tile([128, H, 128], BF16, name="kT_sb")
            nc.vector.memset(kT_sb, 0.0)
            for g in range(4):
                kT_ps = psq.tile([128, 128], F32, tag="qx", name="kT_ps")
                for q in range(2):
                    h = 2 * g + q
                    off = q * 64
                    for ct in range(6):
                        nc.tensor.matmul(
                            kT_ps[off : off + 64, :],
                            lhsT=wk_sb[:, ct, h * 64 : (h + 1) * 64],
                            rhs=cT_sb[:, ct, :],
                            start=(ct == 0),
                            stop=(ct == 5),
                        )
                for q in range(2):
                    h = 2 * g + q
                    off = q * 64
                    nc.scalar.activation(
                        kT_sb[off : off + 64, h, :SKV],
                        kT_ps[off : off + 64, :SKV],
                        AF.Identity,
                        bias=bk_hms[off : off + 64, g : g + 1],
                        scale=0.125,
                    )

            # v [kv, d] with rows >= 77 exactly 0 (zero cT cols + e0[:, :77])
            v_ps = psq.tile([128, D], F32, tag="out", name="v_ps")
            for ct in range(6):
                nc.tensor.matmul(
                    v_ps,
                    lhsT=cT_sb[:, ct, :],
                    rhs=wv_sb[:, ct, :],
                    start=(ct == 0),
                    stop=False,
                )
            nc.tensor.matmul(v_ps, lhsT=e0, rhs=bv_pad, start=False, stop=True)
            # rows >= 77 got bv from the rank-1 bias matmul; zero them (they
            # must mask the exp(0)=1 padding rows in the PV contraction).
            v_sb = prep.tile([128, D], BF16, name="v_sb")
            nc.vector.memset(v_sb[64:128, :], 0.0)
            nc.vector.tensor_scalar_mul(v_sb[:SKV, :], v_ps[:SKV, :], SC_V)

            # ---------------- main loop (software-pipelined) ----------
            # Emission order per group i:
            #   load x(i+1) | transposes(i) | qproj(i) | scores+exp(i)
            #   | outproj(i-1)+store | sums/attnT/normalize(i)
            # so the PE chews i's projection+scores while the DVE finishes
            # i-1's softmax divide, and outproj(i-1) lands with no PE stall.
            # PSUM tags: "qx" = xT+qT (2x1 bank), "sT" = scores+sums ring
            # (2x1 bank), "attnT" (2 banks), "out" (2x1 bank) -> 8 banks.
            x_tiles = {}

            def load_x(it):
                if it >= n_iters:
                    return
                x_sb = xpool.tile([128, NCH, D], X_DT, tag="x", name="x_sb")
                nc.gpsimd.dma_start(
                    x_sb, lat[it * NROW : (it + 1) * NROW, :].rearrange(
                        "(c p) d -> p c d", p=128
                    )
                )
                x_tiles[it] = x_sb

            def stage_front(it):
                """transposes + qproj + scores + exp for iter it."""
                x_sb = x_tiles.pop(it)
                xT_sb = xpool.tile([128, 4, NROW], X_DT, tag="xT", name="xT_sb")
                if XT_MODE == "xbar":
                    for c in range(NCH):
                        nc.sync.dma_start_transpose(
                            xT_sb[:, :, c * 128 : (c + 1) * 128], x_sb[:, c, :]
                        )
                else:
                    for c in range(NCH):
                        xT_ps = psq.tile(
                            [128, 4, 128], X_DT, tag="qx", name="xT_ps"
                        )
                        for et in range(4):
                            nc.tensor.transpose(
                                xT_ps[:, et, :],
                                x_sb[:, c, et * 128 : (et + 1) * 128],
                                ident_x,
                            )
                        nc.scalar.copy(xT_sb[:, :, c * 128 : (c + 1) * 128], xT_ps)

                qT_sb = spool.tile([128, 4, NROW], BF16, tag="qT", name="qT_sb")
                for g in range(4):
                    qT_ps = psq.tile([128, NROW], F32, tag="qx", name="qT_ps")
                    if QP_FP8:
                        for e2 in range(2):
                            nc.tensor.matmul(
                                qT_ps,
                                lhsT=wq_mm[:, 2 * e2 : 2 * e2 + 2, g * 128 : (g + 1) * 128],
                                rhs=xT_sb[:, 2 * e2 : 2 * e2 + 2, :],
                                start=(e2 == 0),
                                stop=(e2 == 1),
                                perf_mode=DR,
                            )
                    else:
                        for et in range(4):
                            nc.tensor.matmul(
                                qT_ps,
                                lhsT=wq_mm[:, et, g * 128 : (g + 1) * 128],
                                rhs=xT_sb[:, et, :],
                                start=(et == 0),
                                stop=(et == 3),
                            )
                    nc.vector.tensor_scalar(
                        qT_sb[:, g, :], qT_ps, SC_Q, bq_pair[:, g : g + 1],
                        ALU.mult, ALU.add,
                    )

                expT_sb = spool.tile([128, H, NROW], BF16, tag="expT", name="expT_sb")
                for g in range(4):
                    sT_ps = pss.tile([128, 2, NROW], F32, tag="sT", name="sT_ps")
                    for q in range(2):
                        nc.tensor.matmul(
                            sT_ps[:, q, :],
                            lhsT=kT_sb[:, 2 * g + q, :],
                            rhs=qT_sb[:, g, :],
                            start=True,
                            stop=True,
                        )
                    nc.scalar.activation(
                        expT_sb[:, 2 * g : 2 * g + 2, :], sT_ps, AF.Exp
                    )
                return expT_sb

            def stage_back(it, expT_sb):
                """sums + PV + softmax normalize for iter it."""
                attnT_ps = psw.tile([128, 4, NROW], F32, tag="attnT", name="attnT_ps")
                attnT_sb = spool.tile([128, 4, NROW], A_DT, tag="attnT", name="attnT_sb")
                for half in range(2):
                    sums_ps = pss.tile([128, 2, NROW], F32, tag="sT", name="sums_ps")
                    for hh in range(4):
                        h = half * 4 + hh
                        dt, off = hh // 2, (h % 2) * 64
                        nc.tensor.matmul(
                            sums_ps[off : off + 64, dt, :],
                            lhsT=ones64,
                            rhs=expT_sb[:, h, :],
                            start=True,
                            stop=True,
                        )
                    for hh in range(4):
                        h = half * 4 + hh
                        dt, off = half * 2 + hh // 2, (h % 2) * 64
                        nc.tensor.matmul(
                            attnT_ps[off : off + 64, dt, :],
                            lhsT=v_sb[:, h * 64 : (h + 1) * 64],
                            rhs=expT_sb[:, h, :],
                            start=True,
                            stop=True,
                        )
                    asb = attnT_sb[:, 2 * half : 2 * half + 2, :]
                    aps = attnT_ps[:, 2 * half : 2 * half + 2, :]
                    if USE_DIV:
                        nc.vector.tensor_tensor(asb, aps, sums_ps, ALU.divide)
                    else:
                        rsum_sb = spool.tile(
                            [128, 2, NROW], F32, tag="rsum", name="rsum_sb"
                        )
                        nc.vector.reciprocal_approx_fast(rsum_sb, sums_ps)
                        nc.vector.tensor_mul(asb, aps, rsum_sb)
                return attnT_sb

            def stage_out(it, attnT_sb):
                """out projection + bias + store for iter it."""
                for c in range(NCH):
                    out_ps = psq.tile([128, D], F32, tag="out", name="out_ps")
                    if OP_FP8:
                        for d2 in range(2):
                            nc.tensor.matmul(
                                out_ps,
                                lhsT=attnT_sb[:, 2 * d2 : 2 * d2 + 2, c * 128 : (c + 1) * 128],
                                rhs=wo_mm[:, 2 * d2 : 2 * d2 + 2, :],
                                start=(d2 == 0),
                                stop=(d2 == 1),
                                perf_mode=DR,
                            )
                    else:
                        for dt in range(4):
                            nc.tensor.matmul(
                                out_ps,
                                lhsT=attnT_sb[:, dt, c * 128 : (c + 1) * 128],
                                rhs=wo_mm[:, dt, :],
                                start=(dt == 0),
                                stop=(USE_GPS and dt == 3),
                            )
                    out_sb = opool.tile([128, D], F32, tag="out", name="out_sb")
                    if USE_GPS:
                        nc.gpsimd.tensor_add(out_sb, out_ps, bo_bcast)
                    else:
                        nc.tensor.matmul(
                            out_ps, lhsT=e0, rhs=bo_pad, start=False, stop=True
                        )
                        nc.scalar.copy(out_sb, out_ps)
                    nc.sync.dma_start(
                        out_d[it * NROW + c * 128 : it * NROW + (c + 1) * 128, :],
                        out_sb,
                    )

            load_x(0)
            load_x(1)
            prev = None
            for it in range(n_iters):
                load_x(it + 2)
                expT = stage_front(it)
                if prev is not None:
                    stage_out(it - 1, prev)
                prev = stage_back(it, expT)
            stage_out(n_iters - 1, prev)

    nc.compile()
    return nc


_BUILD_CACHE = {}


def _get_nc():
    key = (QP_FP8, OP_FP8, XT_MODE, USE_DIV)
    if key not in _BUILD_CACHE:
        _BUILD_CACHE[key] = build_nc()
    return _BUILD_CACHE[key]


def _in_maps(latent, context, wq, bq, wk, bk, wv, bv, wo, bo):
    f = lambda a: np.ascontiguousarray(np.asarray(a), dtype=np.float32)
    shared = {
        "wq": f(wq), "bq": f(bq), "wk": f(wk), "bk": f(bk),
        "wv": f(wv), "bv": f(bv), "wo": f(wo), "bo": f(bo),
    }
    maps = []
    for b in range(N_CORES):
        m = dict(shared)
        m["latent"] = f(latent[b])
        m["context"] = f(context[b])
        maps.append(m)
    return maps


def run_on_hw(inputs, trace=False, **kw):
    nc = _get_nc()
    maps = _in_maps(**inputs)
    res = run_bass_kernel_spmd(nc, maps, list(range(N_CORES)), trace=trace, **kw)
    out = np.stack([res.results[b]["out"] for b in range(N_CORES)], axis=0)
    return out, res


def kernel(latent, context, wq, bq, wk, bk, wv, bv, wo, bo):
    out, _ = run_on_hw(dict(
        latent=latent, context=context, wq=wq, bq=bq, wk=wk, bk=bk,
        wv=wv, bv=bv, wo=wo, bo=bo,
    ))
    return out
